# revision 13
# baseline (speedup 1.0000x reference)
"""Trainium2 Bass kernel: BFP (block-floating-point) activation quantization.

Reference semantics (input NCHW [32, 256, 56, 56] f32):
  per (batch, pixel), channels grouped in blocks of 32:
    maxabs = max |x| over the block
    e      = floor(log2(maxabs))          (guard zero blocks)
    s      = 2^(e-4)                      (5-bit mantissa, QMAX = 31)
    out    = clip(round_half_even(x / s), -31, 31) * s    (0 if maxabs == 0)

Implementation (bit-exact in fp32, validated against the reference):
  The whole mask+clip+round+rescale runs as ONE fused DVE op per element:
      e   = maxabs & 0x7F800000          (= 2^floor(log2(maxabs)) as f32)
      m   = e * 1.9375                   (= 31 * s,  s = 2^(e-4))
      y   = min(max(x, 0 - m), m)        (clip first — proven equal to the
                                          reference's round-then-clip at all
                                          half-even boundaries)
      C   = e * 786432.0                 (= 1.5*2^23 * s magic constant)
      out = (y + C) - C                  (round-half-even to a multiple of s)
  Every step is exact in fp32; outputs are +-q * 2^(e-4), q <= 31 (5
  significant bits), hence exactly representable in bf16 — the kernel stores
  bf16 and the host widens to f32 losslessly, halving store HBM traffic.
  The AND mask comes in through the per-partition constant slot as +inf
  (bit pattern 0x7F800000), memset as an integer to dodge non-finite float
  immediates in BIR serialization.

Layout: everything runs in the natural NCHW layout (channels on SBUF
partitions).  The cross-partition block-of-32 reduction uses the DVE's
32x32 stream-transpose front-end twice:
  1. tensor_reduce(apply_transpose=True) on x [128, HW/32, 32] reduces the
     transposed 32x32 blocks along X, i.e. across the 32 partitions of each
     channel block: mm[32P+i, g] = max_j |x[32P+j, 32g+i]|.
  2. The fused quantize op runs with transpose_mode=TRANSPOSE on SRC_0: the
     raw block maxes mm (small, [128, HW/32]) stream through the same
     front-end with a stride-0 inner broadcast, which lands maxabs(block P,
     pixel f) on every lane of block P at stream position f — aligned with
     SRC_1 = x streamed naturally.  Output writes bf16 in natural layout.
No tensor-engine transposes, no PSUM, no scalar-engine copies: two DVE
passes over the data + DMA.  The DVE runs at ~1 elem/lane/cycle (0.96 GHz),
so the kernel sits right at the two-pass DVE / HBM boundary.

Sharding: batch 32 -> 4 per core across 8 NeuronCores; no cross-core comms.
"""

import numpy as np

import concourse.bass as bass
import concourse.mybir as mybir
from concourse import bacc, tile
from concourse.bass_utils import run_bass_kernel_spmd

F32 = mybir.dt.float32
BF16 = mybir.dt.bfloat16
I32 = mybir.dt.int32

_OP_NAME = "BFP_Q5F_ANT"
_EXP_MASK = 0x7F800000


def _bfp_q5f_reference(in0, in1, s0, s1, imm2):
    # Models the hardware: SRC_0's element stream passes through the 32x32
    # transpose reorder array before the ALU body; SRC_1 streams naturally.
    # s0 arrives as the per-partition constant (+inf = the exponent mask),
    # s1 = 786432.0 (magic), imm2 = 1.9375 (clip scale).
    p = in0.shape[0]
    a = np.asarray(in0, np.float32).reshape(p, -1)
    x = np.asarray(in1, np.float32).reshape(p, -1)
    a4 = a.reshape(p // 32, 32, a.shape[1] // 32, 32)
    t = np.ascontiguousarray(a4.transpose(0, 3, 2, 1)).reshape(p, -1)
    mask = np.asarray(s0, np.float32).reshape(-1, 1).view(np.int32)
    e = (t.view(np.int32) & mask).view(np.float32)
    m = (e * np.float32(imm2)).astype(np.float32)
    c = (e * np.float32(s1)).astype(np.float32)
    y = np.minimum(np.maximum(x, (np.float32(0.0) - m).astype(np.float32)), m)
    return ((y + c).astype(np.float32) - c).astype(np.float32)


def _register_custom_op():
    import concourse.dve_ops as dve_ops
    from concourse.dve_ops import DveOp, _COMPILE_CACHE
    from concourse.dve_spec import (
        C0, C1, C2, Bin, Spec, Src0, Src1, Zero, lower, maxx, minn,
    )
    from concourse.dve_uop import AluOp, DveOpSpec, OpConfig, TransposeMode

    for op in dve_ops.OPS:
        if op.name == _OP_NAME:
            return op

    e = Bin(AluOp.BITWISE_AND, Src0, C0)   # C0 = +inf (exp mask), per-partition
    m = e * C2                             # C2 = imm2 = 1.9375 -> 31*s
    y = minn(maxx(Src1, Zero - m), m)      # clip
    c = e * C1                             # C1 = 786432.0 -> magic
    spec = Spec(
        body=(y + c) - c,
        reference=_bfp_q5f_reference,
    )
    row = dve_ops._CUSTOM_DVE_ROW_BASE + len(dve_ops.OPS)
    ocfg = OpConfig(transpose_mode=TransposeMode.TRANSPOSE)
    shas = {}
    compiled = {}
    for ver in ("v3", "v4"):
        s = DveOpSpec(
            name=_OP_NAME, opcode=row, uops=lower(spec, ver=ver),
            rd1_en=True, op=ocfg,
        )
        s.validate(ver)
        compiled[ver] = s
        shas[ver] = s.sha(ver)
    op = DveOp(_OP_NAME, spec, subdim=False, uops_sha=shas)
    dve_ops.OPS.append(op)
    dve_ops.CUSTOM_DVE_SPECS[_OP_NAME] = spec
    dve_ops._SUB_OPCODE_FOR_NAME[_OP_NAME] = row
    # compile() consults this cache first; seeding it carries the OpConfig
    # (transpose_mode) into the per-NEFF DVE table.
    for ver, s in compiled.items():
        _COMPILE_CACHE[(_OP_NAME, ver)] = s
    return op


# ---------------------------------------------------------------------------
# Tile kernel (per core): x [4, 256, 3136] f32 -> y [4, 256, 3136] bf16
# ---------------------------------------------------------------------------
B_PER_CORE = 4
C_CH = 256
HW = 3136          # 56*56


def bfp_tile_kernel(ctx, tc, y_ap, x_ap):
    nc = tc.nc
    op = _register_custom_op()

    const_pool = ctx.enter_context(tc.tile_pool(name="const", bufs=1))
    x_pool = ctx.enter_context(tc.tile_pool(name="xin", bufs=5))
    q_pool = ctx.enter_context(tc.tile_pool(name="qsb", bufs=3))
    m_pool = ctx.enter_context(tc.tile_pool(name="msb", bufs=4))

    c_inf = const_pool.tile([128, 1], F32, name="c_inf")
    nc.gpsimd.memset(c_inf[:].bitcast(I32), _EXP_MASK)

    # (b, h, px0, npx) chunks; the first and last (b,h) rows are split into
    # quarters (sizes divisible by 32) so the DVE starts on a small first
    # load and the final stores overlap the last quantizes.
    ramp = [896, 1472, 768]
    jobs = []
    for b in range(B_PER_CORE):
        for h in range(2):
            if (b, h) == (0, 0) or (b, h) == (B_PER_CORE - 1, 1):
                px0 = 0
                for npx in ramp:
                    jobs.append((b, h, px0, npx))
                    px0 += npx
            else:
                jobs.append((b, h, 0, HW))

    x_sbs = {}

    def emit_load(b, h, px0, npx):
        x_sb = x_pool.tile([128, npx], F32, tag=f"x{npx}",
                           name=f"x_{b}_{h}_{px0}")
        nc.sync.dma_start(
            out=x_sb[:],
            in_=x_ap[b, h * 128:(h + 1) * 128, px0:px0 + npx],
        )
        x_sbs[(b, h, px0)] = x_sb

    prefetch = 0
    for i, (b, h, px0, npx) in enumerate(jobs):
        while prefetch < len(jobs) and prefetch <= i + 4:
            emit_load(*jobs[prefetch])
            prefetch += 1
        x_sb = x_sbs.pop((b, h, px0))
        ng = npx // 32

        mm = m_pool.tile([128, ng], F32, tag=f"mm{npx}",
                         name=f"mm_{b}_{h}_{px0}")
        nc.vector.tensor_reduce(
            out=mm[:],
            in_=x_sb[:].rearrange("p (g k) -> p g k", k=32),
            axis=mybir.AxisListType.X,
            op=mybir.AluOpType.max,
            apply_absolute_value=True,
            apply_transpose=True,
        )
        q = q_pool.tile([128, npx], BF16, tag=f"q{npx}",
                        name=f"q_{b}_{h}_{px0}")
        nc.vector._custom_dve(
            op,
            out=q[:],
            in0=mm[:].unsqueeze(-1).broadcast_to([128, ng, 32]),
            in1=x_sb[:],
            s0=c_inf[:],
            s1=786432.0,
            imm2=1.9375,
        )
        nc.scalar.dma_start(
            out=y_ap[b, h * 128:(h + 1) * 128, px0:px0 + npx],
            in_=q[:],
        )


# ---------------------------------------------------------------------------
# Build + run
# ---------------------------------------------------------------------------
_CACHED = {}


def build_bass(n_cores=8):
    from contextlib import ExitStack

    nc = bacc.Bacc(
        "TRN2",
        target_bir_lowering=False,
        debug=False,
        enable_asserts=False,
        num_devices=n_cores,
    )
    x = nc.dram_tensor("activations", [B_PER_CORE, C_CH, HW], F32,
                       kind="ExternalInput").ap()
    y = nc.dram_tensor("out", [B_PER_CORE, C_CH, HW], BF16,
                       kind="ExternalOutput").ap()
    with tile.TileContext(nc) as tc:
        with ExitStack() as ctx:
            bfp_tile_kernel(ctx, tc, y, x)
    nc.compile()
    return nc


def kernel(activations: np.ndarray) -> np.ndarray:
    x = np.ascontiguousarray(np.asarray(activations), dtype=np.float32)
    B, C, H, W = x.shape            # [32, 256, 56, 56]
    n_cores = 8
    bpc = B // n_cores              # 4
    xs = x.reshape(n_cores, bpc, C, H * W)
    in_maps = [{"activations": np.ascontiguousarray(xs[c])} for c in range(n_cores)]

    if "nc" not in _CACHED:
        _CACHED["nc"] = build_bass(n_cores)
    nc = _CACHED["nc"]

    res = run_bass_kernel_spmd(nc, in_maps, core_ids=list(range(n_cores)))
    out = np.stack([np.asarray(res.results[c]["out"]) for c in range(n_cores)])
    return out.reshape(B, C, H, W).astype(np.float32)
